# revision 1
# baseline (speedup 1.0000x reference)
"""Trainium2 Bass kernel for a dual cross-attention block.

Computes, per batch element b (8 total, one per NeuronCore):
    Q  = obj @ Wq.T + bq                    [2048, 1024]
    Kx = x @ Wxk.T + bxk,  Vx = x @ Wxv.T + bxv   for x in {sub, scene}
    Ix = LayerNorm(obj + softmax(Q Kx.T / 32) Vx)  -> (I1, I2)

Design:
  - data-parallel over batch: core c handles batch element c (no collectives)
  - all matmuls run as fp8(e4m3) DoubleRow (2 contraction rows per PE pass);
    activations/weights are quantized host-side (weights pre-scaled x64 for
    fp8 range), Q/K stored on-chip at 4x scale, V at 1x; the scale factors
    fold into the exp() scale and the PSUM->SBUF conversions
  - softmax denominator d comes from an fp8 ones-column matmul sharing the
    PV stationary operand; instead of dividing by d, the residual is scaled:
    LN(obj + O/d) == LN(d*obj + O) with the eps correction applied as
    eps*d^2 (exact identity; LN is scale-invariant per row)
  - V-projection bias is folded into the obj residual uploads (rank-1
    identity: P@(V + 1 b^T) = P@V + d b^T, and d*obj + d*b^T = d*(obj+b^T));
    the x' = d*obj + O pass drains each PSUM accumulation in one DVE op
    that also emits Sum(x) for the mean
  - LN stats via DVE bn_stats over the bf16 x' (variance exact in fp32
    aggregation); rstd = exp(-0.5*ln(var + eps*d^2)) keeps every activation
    in ONE act-func table, pinned by an explicit InstLoadActFuncSet (no
    table-reload churn on ACT)
  - the five projections, Q-chunk projections, scores, PV and the LN
    epilogues are software-pipelined in one woven emission order so PE,
    ACT, DVE and Pool all stay busy from ~15us onward
"""

import numpy as np
import ml_dtypes

SQ = 2048
SKV = 1024
EMB = 1024
PROJ = 1024
NCORES = 8
EPS = 1e-5
SCALE = PROJ ** -0.5

WS = 64.0   # weight upload scale (fp8 range)
QS = 4.0    # Q/K on-chip storage scale
EC = EMB // 128   # 8 contraction chunks of 128
NPAIR = EC // 2   # 4 DoubleRow pairs

_CACHE = {}
LAST_RESULTS = None

import os as _os
K1DVE = _os.environ.get("K1DVE", "0") == "1"   # K1 convs on DVE (else ACT)
K2DVE = _os.environ.get("K2DVE", "1") == "1"   # K2 convs on DVE (else ACT)
QDVE = int(_os.environ.get("QDVE", "0"))       # of 2 Q convs per pair on DVE
BNMOD = int(_os.environ.get("BNMOD", "1"))     # bn_stats when c % BNMOD == 0
NMRACT = _os.environ.get("NMRACT", "1") == "1"
UNITWEAVE = _os.environ.get("UNITWEAVE", "1") == "1"


def _build(skip_gb=False):
    import concourse.bass as bass
    import concourse.tile as tile
    import concourse.mybir as mybir
    from concourse import bacc
    from concourse.hw_specs import get_activation_tables

    dt = mybir.dt
    f32 = dt.float32
    bf16 = dt.bfloat16
    f8 = dt.float8e4
    Act = mybir.ActivationFunctionType
    Alu = mybir.AluOpType
    DR = mybir.MatmulPerfMode.DoubleRow

    nc = bacc.Bacc("TRN2", debug=False)

    # one activation table covering every function we use
    tables = list(get_activation_tables(nc.m.arch).items())
    need = {Act.Exp, Act.Ln, Act.Identity, Act.Square}
    act_set_id = next(i for i, (_, s) in enumerate(tables) if need <= s)

    # ---- DRAM I/O ----
    objT_d = nc.dram_tensor("objT", [SQ // 512, 128, EC, 512], f8,
                            kind="ExternalInput")
    subT_d = nc.dram_tensor("subT", [SKV // 512, 128, EC, 512], f8,
                            kind="ExternalInput")
    scnT_d = nc.dram_tensor("scnT", [SKV // 512, 128, EC, 512], f8,
                            kind="ExternalInput")
    w_d = {
        n: nc.dram_tensor(f"W{n}T", [NPAIR, 128, 2, PROJ], f8,
                          kind="ExternalInput")
        for n in ["q", "sk", "ek", "sv", "ev"]
    }
    objr1_d = nc.dram_tensor("objr1", [SQ // 128, 128, EMB], bf16,
                             kind="ExternalInput")
    objr2_d = nc.dram_tensor("objr2", [SQ // 128, 128, EMB], bf16,
                             kind="ExternalInput")
    # per-row sums of objr1/objr2 (column j = row block j) and the identity
    objr1s_d = nc.dram_tensor("objr1s", [128, SQ // 128], f32,
                              kind="ExternalInput")
    objr2s_d = nc.dram_tensor("objr2s", [128, SQ // 128], f32,
                              kind="ExternalInput")
    ident_d = nc.dram_tensor("ident", [128, 128], bf16,
                             kind="ExternalInput")
    bq_d = nc.dram_tensor("bq", [128, EC], f32, kind="ExternalInput")
    bsk_d = nc.dram_tensor("bsk", [128, EC], f32, kind="ExternalInput")
    bek_d = nc.dram_tensor("bek", [128, EC], f32, kind="ExternalInput")
    G_d = nc.dram_tensor("G", [128, EMB], bf16, kind="ExternalInput")
    B_d = nc.dram_tensor("Bb", [128, EMB], bf16, kind="ExternalInput")
    I1_d = nc.dram_tensor("I1", [SQ // 128, 128, EMB], bf16,
                          kind="ExternalOutput")
    I2_d = nc.dram_tensor("I2", [SQ // 128, 128, EMB], bf16,
                          kind="ExternalOutput")

    with tile.TileContext(nc) as tc:
        with (
            tc.tile_pool(name="const", bufs=1) as cpool,
            tc.tile_pool(name="kv", bufs=1) as kv,
            tc.tile_pool(name="wp", bufs=1) as wp,
            tc.tile_pool(name="src", bufs=1) as srcp,
            tc.tile_pool(name="etp", bufs=1) as etp,
            tc.tile_pool(name="epi", bufs=1) as epi,
            tc.tile_pool(name="smp", bufs=1) as smp,
            tc.tile_pool(name="pp", bufs=1, space="PSUM") as pp,
            tc.tile_pool(name="opp", bufs=1, space="PSUM") as opp,
            tc.tile_pool(name="dpp", bufs=1, space="PSUM") as dpp,
        ):
            nc.scalar.add_instruction(mybir.InstLoadActFuncSet(
                name=nc.get_next_instruction_name(),
                act_func_set_id=act_set_id, ins=[], outs=[]))

            ones2 = cpool.tile([128, 2, 16], f8, name="ones2")
            nc.vector.memset(ones2, 1.0)
            vsum1 = vsum2 = so1_s = so2_s = None

            # resident fp8 operands for the attention phase
            kt1 = kv.tile([128, EC, SKV], f8, name="kt1")
            kt2 = kv.tile([128, EC, SKV], f8, name="kt2")
            vt1 = kv.tile([128, EC, PROJ], f8, name="vt1")
            vt2 = kv.tile([128, EC, PROJ], f8, name="vt2")
            qt = kv.tile([128, EC, SQ], f8, name="qt")

            # weight chunks; DMA issue order tracks consumption order so the
            # serialized DMA engines deliver operands just in time
            wch = {}
            qi = [0]

            def load_w(n):
                wch[n] = []
                for i in range(NPAIR):
                    t = wp.tile([128, 2, PROJ], f8, tag="w", bufs=20,
                                name=f"w{n}{i}")
                    eng = (nc.sync, nc.gpsimd)[qi[0] % 2]
                    qi[0] += 1
                    eng.dma_start(t, w_d[n][i])
                    wch[n].append(t)

            def load_src(dram, tag, q0):
                ts = []
                for c in range(SKV // 512):
                    t = srcp.tile([128, EC, 512], f8, name=f"{tag}{c}")
                    eng = (nc.sync, nc.gpsimd)[(q0 + c) % 2]
                    eng.dma_start(t, dram[c])
                    ts.append(t)
                return ts

            # critical-path first: sub chunk0, wsk, sub chunk1, wsv, wq;
            # consts (scalar queue) only after the gating loads
            sub0 = srcp.tile([128, EC, 512], f8, name="sub0")
            nc.sync.dma_start(sub0, subT_d[0])
            load_w("sk")
            sub1 = srcp.tile([128, EC, 512], f8, name="sub1")
            nc.gpsimd.dma_start(sub1, subT_d[1])
            sub_t = [sub0, sub1]
            load_w("sv")
            load_w("q")
            bq_s = cpool.tile([128, EC], f32, name="bq_s")
            nc.scalar.dma_start(bq_s, bq_d[:, :])
            bsk_s = cpool.tile([128, EC], f32, name="bsk_s")
            nc.scalar.dma_start(bsk_s, bsk_d[:, :])
            bek_s = cpool.tile([128, EC], f32, name="bek_s")
            nc.scalar.dma_start(bek_s, bek_d[:, :])
            G_s = cpool.tile([128, EMB], bf16, name="G_s")
            nc.scalar.dma_start(G_s, G_d[:, :])
            B_s = cpool.tile([128, EMB], bf16, name="B_s")
            nc.scalar.dma_start(B_s, B_d[:, :])
            scn_t = load_src(scnT_d, "scn", 1)

            # single-bank denominator PSUM: two independent accumulation
            # column-pairs (per-element has_written) alternate per subtile
            den2 = dpp.tile([128, 8], f32, name="den2")

            # ---------- emission helpers (shared PSUM rotation) ----------
            def proj_K(src, wc, bias, KT, conv_dve):
                """KT[p_lo, pc, k] = QS*(x @ Wk.T + b)[k, p].T"""
                for pc in range(EC):
                    for kn in range(SKV // 512):
                        ps = pp.tile([128, 512], f32, tag="ps512", bufs=3,
                                     name="ps_k")
                        for i in range(NPAIR):
                            nc.tensor.matmul(
                                ps,
                                wc[i][:, :, pc * 128:(pc + 1) * 128],
                                src[kn][:, 2 * i:2 * i + 2, :],
                                start=(i == 0), stop=(i == NPAIR - 1),
                                perf_mode=DR,
                            )
                        dst = KT[:, pc, kn * 512:(kn + 1) * 512]
                        if conv_dve:
                            nc.vector.tensor_scalar(
                                dst, ps, QS / WS, bias[:, pc:pc + 1],
                                op0=Alu.mult, op1=Alu.add)
                        else:
                            nc.scalar.activation(
                                dst, ps, Act.Identity,
                                bias=bias[:, pc:pc + 1], scale=QS / WS)

            def proj_V(src, wc, VT, vsum):
                """VT[k_lo, kc, p] = (x @ Wv.T)[k, p]  (bias folded out);
                also fills vsum[:, kc, 1] with the V row-sums."""
                for kc in range(EC):
                    st = src[kc // 4]
                    k0 = (kc % 4) * 128
                    for po in range(PROJ // 512):
                        ps = pp.tile([128, 512], f32, tag="ps512", bufs=3,
                                     name="ps_v")
                        for i in range(NPAIR):
                            nc.tensor.matmul(
                                ps,
                                st[:, 2 * i:2 * i + 2, k0:k0 + 128],
                                wc[i][:, :, po * 512:(po + 1) * 512],
                                start=(i == 0), stop=(i == NPAIR - 1),
                                perf_mode=DR,
                            )
                        nc.vector.tensor_scalar_mul(
                            VT[:, kc, po * 512:(po + 1) * 512], ps, 1.0 / WS)

            def proj_Q(qc):
                ot = srcp.tile([128, EC, 512], f8, tag="ot", bufs=2,
                               name=f"ot{qc}")
                eng = (nc.sync, nc.gpsimd)[qc % 2]
                eng.dma_start(ot, objT_d[qc])
                for pc in range(EC):
                    ps = pp.tile([128, 512], f32, tag="ps512", bufs=3,
                                 name="ps_q")
                    for i in range(NPAIR):
                        nc.tensor.matmul(
                            ps,
                            wch["q"][i][:, :, pc * 128:(pc + 1) * 128],
                            ot[:, 2 * i:2 * i + 2, :],
                            start=(i == 0), stop=(i == NPAIR - 1),
                            perf_mode=DR,
                        )
                    dst = qt[:, pc, qc * 512:(qc + 1) * 512]
                    if pc % 2 < QDVE:
                        nc.vector.tensor_scalar(
                            dst, ps, QS / WS, bq_s[:, pc:pc + 1],
                            op0=Alu.mult, op1=Alu.add)
                    else:
                        nc.scalar.activation(
                            dst, ps, Act.Identity,
                            bias=bq_s[:, pc:pc + 1], scale=QS / WS)

            def scores_units(KT, qc, tag):
                et = etp.tile([128, EC, 512], f8, tag=tag, bufs=2, name=tag)

                def unit(kc):
                    def f():
                        ps = pp.tile([128, 512], f32, tag="ps512", bufs=3,
                                     name="sps")
                        for i in range(NPAIR):
                            nc.tensor.matmul(
                                ps,
                                KT[:, 2 * i:2 * i + 2,
                                   kc * 128:(kc + 1) * 128],
                                qt[:, 2 * i:2 * i + 2,
                                   qc * 512:(qc + 1) * 512],
                                start=(i == 0), stop=(i == NPAIR - 1),
                                perf_mode=DR,
                            )
                        nc.scalar.activation(et[:, kc, :], ps, Act.Exp,
                                             scale=SCALE / (QS * QS))
                    return f
                return et, [unit(kc) for kc in range(EC)]

            def scores(KT, qc, tag):
                et, units = scores_units(KT, qc, tag)
                for u in units:
                    u()
                return et

            def proj_K_units(src_, wc, bias, KT, conv_dve):
                def unit(pc, kn):
                    def f():
                        ps = pp.tile([128, 512], f32, tag="ps512", bufs=3,
                                     name="ps_k")
                        for i in range(NPAIR):
                            nc.tensor.matmul(
                                ps,
                                wc[i][:, :, pc * 128:(pc + 1) * 128],
                                src_[kn][:, 2 * i:2 * i + 2, :],
                                start=(i == 0), stop=(i == NPAIR - 1),
                                perf_mode=DR,
                            )
                        dst = KT[:, pc, kn * 512:(kn + 1) * 512]
                        if conv_dve:
                            nc.vector.tensor_scalar(
                                dst, ps, QS / WS, bias[:, pc:pc + 1],
                                op0=Alu.mult, op1=Alu.add)
                        else:
                            nc.scalar.activation(
                                dst, ps, Act.Identity,
                                bias=bias[:, pc:pc + 1], scale=QS / WS)
                    return f
                return [unit(pc, kn) for pc in range(EC)
                        for kn in range(SKV // 512)]

            def proj_Q_units(qc):
                ot = srcp.tile([128, EC, 512], f8, tag="ot", bufs=2,
                               name=f"otu{qc}")
                eng = (nc.sync, nc.gpsimd)[qc % 2]
                eng.dma_start(ot, objT_d[qc])

                def unit(pc):
                    def f():
                        ps = pp.tile([128, 512], f32, tag="ps512", bufs=3,
                                     name="ps_q")
                        for i in range(NPAIR):
                            nc.tensor.matmul(
                                ps,
                                wch["q"][i][:, :, pc * 128:(pc + 1) * 128],
                                ot[:, 2 * i:2 * i + 2, :],
                                start=(i == 0), stop=(i == NPAIR - 1),
                                perf_mode=DR,
                            )
                        dst = qt[:, pc, qc * 512:(qc + 1) * 512]
                        if pc % 2 < QDVE:
                            nc.vector.tensor_scalar(
                                dst, ps, QS / WS, bq_s[:, pc:pc + 1],
                                op0=Alu.mult, op1=Alu.add)
                        else:
                            nc.scalar.activation(
                                dst, ps, Act.Identity,
                                bias=bq_s[:, pc:pc + 1], scale=QS / WS)
                    return f
                return [unit(pc) for pc in range(EC)]

            ecnt = [0]

            def pv_block(et, VT, vsum, so_s, ob, out_d, qc, qs):
                """PV + denominator matmuls; x' = d*obj + O drains the PSUM
                immediately (one DVE pass, also yields Sum(x)); stats via
                bn_stats (DVE) or Square+accum (ACT) per BNMOD."""
                idx = qc * 4 + qs
                q0 = qs * 128
                c = ecnt[0]
                ecnt[0] += 1
                ops = opp.tile([128, PROJ], f32, tag="ops", bufs=2,
                               name="ops")
                dc = (c % 4) * 2
                den = den2[:, dc:dc + 1]
                for i in range(NPAIR):
                    stat = et[:, 2 * i:2 * i + 2, q0:q0 + 128]
                    nc.tensor.matmul(
                        den, stat, ones2[:, :, 0:1],
                        start=(i == 0), stop=(i == NPAIR - 1),
                        perf_mode=DR,
                    )
                    for po in range(PROJ // 512):
                        nc.tensor.matmul(
                            ops[:, po * 512:(po + 1) * 512],
                            stat,
                            VT[:, 2 * i:2 * i + 2, po * 512:(po + 1) * 512],
                            start=(i == 0), stop=(i == NPAIR - 1),
                            perf_mode=DR,
                        )
                d_col = smp.tile([128, 1], f32, tag="sm", bufs=64,
                                 name="d_col")
                nc.vector.tensor_copy(d_col, den)
                # x' = d*obj + O  (drains PSUM, accumulates Sum(x))
                x = epi.tile([128, EMB], bf16, tag="x", bufs=6, name="x")
                sums = smp.tile([128, 1], f32, tag="sm", bufs=64,
                                name="sums")
                nc.vector.scalar_tensor_tensor(
                    x, ob, d_col, ops, op0=Alu.mult, op1=Alu.add,
                    accum_out=sums)
                epsd2 = smp.tile([128, 1], f32, tag="sm", bufs=64,
                                 name="epsd2")
                nc.vector.scalar_tensor_tensor(
                    epsd2, d_col, EPS, d_col, op0=Alu.mult, op1=Alu.mult)
                nmu = smp.tile([128, 1], f32, tag="sm", bufs=64, name="nmu")
                var = smp.tile([128, 1], f32, tag="sm", bufs=64, name="var")
                if BNMOD > 0 and c % BNMOD == 0 and c < 20:
                    # DVE stats: bn_stats over the SBUF x'
                    bst = smp.tile([128, 2, 6], f32, tag="sm", bufs=64,
                                   name="bst")
                    nc.vector.bn_stats(bst[:, 0:1, :], x[:, 0:512])
                    nc.vector.bn_stats(bst[:, 1:2, :], x[:, 512:1024])
                    mv = smp.tile([128, 2], f32, tag="sm", bufs=64,
                                  name="mv")
                    nc.vector.bn_aggr(mv, bst)
                    nc.vector.tensor_scalar_mul(nmu, mv[:, 0:1], -1.0)
                    nc.vector.tensor_add(var, mv[:, 1:2], epsd2)
                else:
                    # ACT stats: Square + accum
                    ssq = smp.tile([128, 1], f32, tag="sm", bufs=64,
                                   name="ssq")
                    scr = epi.tile([128, EMB], bf16, tag="scr2", bufs=3,
                                   name="scr2")
                    nc.scalar.activation(scr, x, Act.Square, accum_out=ssq)
                    nc.vector.tensor_scalar_mul(nmu, sums, -1.0 / EMB)
                    msqu = smp.tile([128, 1], f32, tag="sm", bufs=64,
                                    name="msqu")
                    nc.vector.tensor_mul(msqu, sums, sums)
                    v1 = smp.tile([128, 1], f32, tag="sm", bufs=64,
                                  name="v1")
                    nc.vector.scalar_tensor_tensor(
                        v1, ssq, 1.0 / EMB, epsd2, op0=Alu.mult, op1=Alu.add)
                    nc.vector.scalar_tensor_tensor(
                        var, msqu, -1.0 / (EMB * EMB), v1, op0=Alu.mult,
                        op1=Alu.add)
                # rstd = exp(-0.5*ln(var)); nmr = -mu*rstd  (ACT-internal)
                lnv = smp.tile([128, 1], f32, tag="sm", bufs=64, name="lnv")
                nc.scalar.activation(lnv, var, Act.Ln)
                rstd = smp.tile([128, 1], f32, tag="sm", bufs=64,
                                name="rstd")
                nc.scalar.activation(rstd, lnv, Act.Exp, scale=-0.5)
                nmr = smp.tile([128, 1], f32, tag="sm", bufs=64, name="nmr")
                if NMRACT:
                    nc.scalar.activation(nmr, rstd, Act.Identity, scale=nmu)
                else:
                    nc.vector.tensor_mul(nmr, rstd, nmu)
                t = epi.tile([128, EMB], bf16, tag="t", bufs=6, name="t")
                nc.scalar.activation(t, x, Act.Identity, bias=nmr,
                                     scale=rstd)
                if skip_gb:
                    # gamma==1, beta==0: t is the final output
                    nc.sync.dma_start(out_d[idx], t)
                else:
                    o1 = epi.tile([128, EMB], bf16, tag="o1", bufs=3,
                                  name="o1")
                    nc.vector.tensor_mul(o1, t, G_s)
                    o = epi.tile([128, EMB], bf16, tag="o", bufs=4, name="o")
                    nc.gpsimd.tensor_add(o, o1, B_s)
                    nc.sync.dma_start(out_d[idx], o)

            def fetch_obs(objr_d, qc):
                obs = []
                for qs in range(4):
                    ob = epi.tile([128, EMB], bf16, tag="ob", bufs=10,
                                  name="ob")
                    nc.sync.dma_start(ob, objr_d[qc * 4 + qs])
                    obs.append(ob)
                return obs

            def block(et, A, qc, units=()):
                VT, vsum, so_s, objr_d, out_d = A
                units = list(units)
                per = (len(units) + 3) // 4 if units else 0
                ui = 0
                obs = fetch_obs(objr_d, qc)
                for qs in range(4):
                    for _ in range(per):
                        if ui < len(units):
                            units[ui]()
                            ui += 1
                    pv_block(et, VT, vsum, so_s, obs[qs], out_d, qc, qs)
                while ui < len(units):
                    units[ui]()
                    ui += 1

            def blockpair(etA, argsA, qcA, etB, argsB, qcB,
                          units=()):
                units = list(units)
                per = (len(units) + 7) // 8 if units else 0
                ui = 0
                obsA = fetch_obs(argsA[3], qcA)
                obsB = fetch_obs(argsB[3], qcB)
                for qs in range(4):
                    for eb, args, obs, qc in ((etA, argsA, obsA, qcA),
                                              (etB, argsB, obsB, qcB)):
                        for _ in range(per):
                            if ui < len(units):
                                units[ui]()
                                ui += 1
                        pv_block(eb, args[0], args[1], args[2], obs[qs],
                                 args[4], qc, qs)
                while ui < len(units):
                    units[ui]()
                    ui += 1

            A1 = (vt1, vsum1, so1_s, objr1_d, I1_d)
            A2 = (vt2, vsum2, so2_s, objr2_d, I2_d)

            # ---------- woven schedule ----------
            proj_K(sub_t, wch["sk"], bsk_s, kt1, conv_dve=K1DVE)
            proj_V(sub_t, wch["sv"], vt1, vsum1)
            proj_Q(0)
            load_w("ek")
            load_w("ev")
            e1_0 = scores(kt1, 0, "et1")
            proj_K(scn_t, wch["ek"], bek_s, kt2, conv_dve=K2DVE)
            block(e1_0, A1, 0)
            proj_Q(1)
            e2_0 = scores(kt2, 0, "et2")
            e1_1 = scores(kt1, 1, "et1")
            proj_V(scn_t, wch["ev"], vt2, vsum2)
            if UNITWEAVE:
                uQ2 = proj_Q_units(2)
                e2_1_et, u21 = scores_units(kt2, 1, "et2")
                e1_2_et, u12 = scores_units(kt1, 2, "et1")
                blockpair(e2_0, A2, 0, e1_1, A1, 1, uQ2 + u21 + u12)
                uQ3 = proj_Q_units(3)
                e2_2_et, u22 = scores_units(kt2, 2, "et2")
                e1_3_et, u13 = scores_units(kt1, 3, "et1")
                blockpair(e2_1_et, A2, 1, e1_2_et, A1, 2, uQ3 + u22 + u13)
                e2_3_et, u23 = scores_units(kt2, 3, "et2")
                blockpair(e2_2_et, A2, 2, e1_3_et, A1, 3, u23)
                block(e2_3_et, A2, 3)
            else:
                proj_Q(2)
                e2_1 = scores(kt2, 1, "et2")
                e1_2 = scores(kt1, 2, "et1")
                blockpair(e2_0, A2, 0, e1_1, A1, 1)
                proj_Q(3)
                e2_2 = scores(kt2, 2, "et2")
                e1_3 = scores(kt1, 3, "et1")
                blockpair(e2_1, A2, 1, e1_2, A1, 2)
                e2_3 = scores(kt2, 3, "et2")
                blockpair(e2_2, A2, 2, e1_3, A1, 3)
                block(e2_3, A2, 3)

    nc.compile()
    return nc


def _prep_in_maps(inputs):
    f8 = ml_dtypes.float8_e4m3
    bf = ml_dtypes.bfloat16
    f = lambda a: np.asarray(a, dtype=np.float32)
    obj = f(inputs["obj"])
    sub = f(inputs["sub"])
    scene = f(inputs["scene"])

    def chunk_xT(xT, width):
        # xT [EMB, S] -> [S//width, 128, 8, width] fp8
        S = xT.shape[1]
        t = xT.reshape(EC, 128, S).transpose(1, 0, 2)  # [128, 8, S]
        t = t.reshape(128, EC, S // width, width).transpose(2, 0, 1, 3)
        return np.ascontiguousarray(t).astype(f8)

    shared = {}
    for n in ["q", "sk", "ek", "sv", "ev"]:
        wt = f(inputs[f"W_{n}"]).T * WS  # [EMB, PROJ]
        t = wt.reshape(EC, 128, PROJ).transpose(1, 0, 2)  # [128, 8, PROJ]
        t = t.reshape(128, NPAIR, 2, PROJ).transpose(1, 0, 2, 3)
        shared[f"W{n}T"] = np.ascontiguousarray(t).astype(f8)
    for key, n in [("bq", "q"), ("bsk", "sk"), ("bek", "ek")]:
        shared[key] = np.ascontiguousarray(
            (f(inputs[f"b_{n}"]) * QS).reshape(EC, 128).T)
    shared["G"] = np.ascontiguousarray(
        np.broadcast_to(f(inputs["ln_g"]), (128, EMB))).astype(bf)
    shared["Bb"] = np.ascontiguousarray(
        np.broadcast_to(f(inputs["ln_b"]), (128, EMB))).astype(bf)
    b_sv = f(inputs["b_sv"])
    b_ev = f(inputs["b_ev"])
    shared["ident"] = np.eye(128, dtype=np.float32).astype(bf)

    in_maps = []
    for b in range(NCORES):
        m = dict(shared)
        m["objT"] = chunk_xT(obj[b].T, 512)
        m["subT"] = chunk_xT(sub[b].T, 512)
        m["scnT"] = chunk_xT(scene[b].T, 512)
        for key, bias in (("objr1", b_sv), ("objr2", b_ev)):
            r = (obj[b] + bias[None, :]).reshape(
                SQ // 128, 128, EMB).astype(bf)
            m[key] = r
            # per-row sums of the bf16-rounded residual, [128, 16]
            m[key + "s"] = np.ascontiguousarray(
                r.astype(np.float32).sum(axis=2).T)
        in_maps.append(m)
    return in_maps


def kernel(**inputs):
    global LAST_RESULTS
    from concourse import bass_utils

    g = np.asarray(inputs["ln_g"], dtype=np.float32)
    b = np.asarray(inputs["ln_b"], dtype=np.float32)
    skip_gb = bool(np.all(g == 1.0) and np.all(b == 0.0))
    key = ("nc", skip_gb)
    if key not in _CACHE:
        _CACHE[key] = _build(skip_gb=skip_gb)
    nc = _CACHE[key]
    in_maps = _prep_in_maps(inputs)
    res = bass_utils.run_bass_kernel_spmd(
        nc, in_maps, core_ids=list(range(NCORES)))
    LAST_RESULTS = res
    I1 = np.stack([
        res.results[c]["I1"].astype(np.float32).reshape(SQ, EMB)
        for c in range(NCORES)])
    I2 = np.stack([
        res.results[c]["I2"].astype(np.float32).reshape(SQ, EMB)
        for c in range(NCORES)])
    return I1, I2

